# revision 1
# baseline (speedup 1.0000x reference)
"""Trainium2 Bass kernel for nn_MetaRL_LightGAT_BiACT (GAT + LayerNorm + MLP).

Strategy (8 NeuronCores, row-sharded):
  - Each core owns 1024 of the 8192 output rows (node dim N).
  - Host precomputes the tiny GAT projection Wh = x @ W_gat.T and the additive
    attention scores s = Wh @ a.T (0.15% of FLOPs), plus weight transposes.
  - adj is cast to int16 on host (values are 0/1) so the 2-byte DMA-transpose
    (xbar) path can load adj^T slabs directly: the kernel works in a
    transposed layout [j_partition, i_free] so the attention aggregation
    matmul (contraction over j) needs NO on-chip PE transposes of the big
    [N,N] intermediates.
  - Per j-superchunk (8 chunks of 128 j's x 1024 i's):
      DVE:   eT = s_j + s_i                (broadcast-AP add)
      ACT:   q  = exp(leaky_relu(eT))      (Lrelu then Exp, same LUT table set)
      DVE/GpSimd: pT = q * adjT            (mask; 0/1 multiply)
      PE:    acc[ib] += pT_chunk^T @ Whaug (Whaug has a ones column so the
                                            softmax denominator accumulates
                                            as column 48 for free)
  - Epilogue: h' = acc[:, :48] / acc[:, 48], LayerNorm (bn_stats/bn_aggr),
    MLP 48->256->128->32 in transposed layout on PE.
"""

import sys

if "/opt/trn_rl_repo" not in sys.path:
    sys.path.insert(0, "/opt/trn_rl_repo")

import numpy as np

N = 8192
D_IN = 128
D_H = 48
D_AUG = 65  # Wh cols 0-47, zeros 48-63, ones col at 64 (quadrant-aligned)
D_OUT = 32
N_CORES = 8
ROWS = N // N_CORES          # 1024 rows per core
P = 128                      # partitions
N_IBLK = ROWS // P           # 8 i-blocks per core
N_CHUNK = N // P             # 64 j-chunks
SC_CHUNKS = 8                # j-chunks per superchunk
N_SC = N_CHUNK // SC_CHUNKS  # 8 superchunks
NEG_SLOPE = 0.2
EPS = 1e-5


def build_nc(num_cores=N_CORES, rows=ROWS, n=N, dve_mask_chunks=8, q_bufs=2,
             adj_bufs=3, lrelu_mode="act", reps=1, add_mode="ts",
             stages="dma,add,act,mask,mm,epi"):
    import concourse.bass as bass
    import concourse.mybir as mybir
    import concourse.tile as tile
    from concourse import bacc
    from concourse.masks import make_identity
    from contextlib import ExitStack

    f32 = mybir.dt.float32
    i16 = mybir.dt.int16
    AF = mybir.ActivationFunctionType
    OP = mybir.AluOpType

    n_iblk = rows // P
    n_chunk = n // P
    n_sc = max(1, n_chunk // SC_CHUNKS)
    sc_chunks = n_chunk // n_sc

    st = {}
    for tok in stages.split(","):
        name, _, mult = tok.partition(":")
        st[name] = int(mult) if mult else 1
    nc = bacc.Bacc("TRN2", target_bir_lowering=False, debug=False,
                   num_devices=num_cores)

    adj16_d = nc.dram_tensor("adj16", [rows, n], i16, kind="ExternalInput").ap()
    whaug_d = nc.dram_tensor("whaug", [n, D_AUG], f32, kind="ExternalInput").ap()
    sP_d = nc.dram_tensor("sP", [P, n_chunk], f32, kind="ExternalInput").ap()
    sI_d = nc.dram_tensor("sI", [1, rows], f32, kind="ExternalInput").ap()
    gamma_d = nc.dram_tensor("gamma", [1, D_H], f32, kind="ExternalInput").ap()
    beta_d = nc.dram_tensor("beta", [1, D_H], f32, kind="ExternalInput").ap()
    w1t_d = nc.dram_tensor("w1t", [D_H, 256], f32, kind="ExternalInput").ap()
    b1_d = nc.dram_tensor("b1", [256, 1], f32, kind="ExternalInput").ap()
    w2t_d = nc.dram_tensor("w2t", [256, 128], f32, kind="ExternalInput").ap()
    b2_d = nc.dram_tensor("b2", [128, 1], f32, kind="ExternalInput").ap()
    w3t_d = nc.dram_tensor("w3t", [128, D_OUT], f32, kind="ExternalInput").ap()
    b3_d = nc.dram_tensor("b3", [D_OUT, 1], f32, kind="ExternalInput").ap()
    out_d = nc.dram_tensor("out", [rows, D_OUT], f32, kind="ExternalOutput").ap()

    with ExitStack() as ctx:
        tc = ctx.enter_context(tile.TileContext(nc))
        singles = ctx.enter_context(tc.tile_pool(name="singles", bufs=1))
        adjp = ctx.enter_context(tc.tile_pool(name="adjp", bufs=adj_bufs))
        qp = ctx.enter_context(tc.tile_pool(name="qp", bufs=q_bufs))
        hp = ctx.enter_context(tc.tile_pool(name="hp", bufs=2))

        # ---- resident small tensors ----
        whaug_sb = singles.tile([P, n_chunk, D_AUG], f32)
        nc.sync.dma_start(whaug_sb, whaug_d.rearrange("(c p) d -> p c d", p=P))
        sP_sb = singles.tile([P, n_chunk], f32)
        nc.sync.dma_start(sP_sb, sP_d)
        sI_sb = singles.tile([P, rows], f32)
        nc.sync.dma_start(sI_sb, sI_d[0:1, :].partition_broadcast(P).rearrange(
            "p one r -> p (one r)"))
        gamma_sb = singles.tile([P, D_H], f32)
        nc.sync.dma_start(gamma_sb, gamma_d[0:1, :].partition_broadcast(P)
                          .rearrange("p one r -> p (one r)"))
        beta_sb = singles.tile([P, D_H], f32)
        nc.sync.dma_start(beta_sb, beta_d[0:1, :].partition_broadcast(P)
                          .rearrange("p one r -> p (one r)"))
        w1t_sb = singles.tile([D_H, 256], f32)
        nc.sync.dma_start(w1t_sb, w1t_d)
        w2t_sb = singles.tile([P, 2, 128], f32)
        nc.sync.dma_start(w2t_sb, w2t_d.rearrange("(m p) k -> p m k", p=P))
        w3t_sb = singles.tile([P, D_OUT], f32)
        nc.sync.dma_start(w3t_sb, w3t_d)
        b1_sb = singles.tile([P, 2], f32)
        nc.sync.dma_start(b1_sb, b1_d.rearrange("(m p) one -> p (m one)", p=P))
        b2_sb = singles.tile([P, 1], f32)
        nc.sync.dma_start(b2_sb, b2_d)
        b3_sb = singles.tile([D_OUT, 1], f32)
        nc.sync.dma_start(b3_sb, b3_d)
        eps_sb = singles.tile([P, 1], f32)
        nc.vector.memset(eps_sb, EPS)
        ident = singles.tile([P, P], f32)
        make_identity(nc, ident)

        def bcast_sb(dst, src_row, parts):
            src = bass.AP(tensor=src_row.tensor, offset=src_row.offset,
                          ap=[src_row.ap[0], [0, parts], src_row.ap[1]])
            dst3 = bass.AP(tensor=dst.tensor, offset=dst.offset,
                           ap=[dst.ap[0], [1, 1], dst.ap[1]])
            nc.sync.dma_start(dst3, src)
        ones48 = singles.tile([D_H, 1], f32)
        nc.vector.memset(ones48, 1.0)
        gammaC = singles.tile([D_H, 1], f32)
        nc.sync.dma_start(gammaC, gamma_d.rearrange("one d -> d one"))
        betaC = singles.tile([D_H, 1], f32)
        nc.sync.dma_start(betaC, beta_d.rearrange("one d -> d one"))

        # ---- main loop: attention aggregation in transposed layout ----
        n_half = rows // 512
        for rep in range(reps):
          with tc.tile_pool(name=f"accp{rep}", bufs=2,
                            space="PSUM") as accp:
            acc = [accp.tile([D_AUG, 512], f32, tag="acc", name=f"acc{i}")
                   for i in range(n_half)]
            for sc in range(n_sc):
                adjT = adjp.tile([P, sc_chunks, rows], i16)
                for _m in range(st.get("dma", 0)):
                    for cc in range(sc_chunks):
                        jc = sc * sc_chunks + cc
                        nc.sync.dma_start(adjT[:, cc, :],
                                          adj16_d[:, jc * P:(jc + 1) * P],
                                          transpose=True)
                q = qp.tile([P, sc_chunks, rows], f32)
                # eT[jp, cc, i] = s_j(jp, sc*8+cc) + s_i(i)
                sj = sP_sb[:, sc * sc_chunks:(sc + 1) * sc_chunks]
                in0 = bass.AP(tensor=sj.tensor, offset=sj.offset,
                              ap=list(sj.ap) + [[0, rows]])
                in1 = bass.AP(tensor=sI_sb.tensor, offset=sI_sb.offset,
                              ap=[sI_sb.ap[0], [0, sc_chunks], sI_sb.ap[1]])
                for _m in range(st.get("add", 0)):
                    if add_mode == "ts":
                        for cc in range(sc_chunks):
                            jc = sc * sc_chunks + cc
                            nc.vector.tensor_scalar(
                                q[:, cc, :], sI_sb, sP_sb[:, jc:jc + 1],
                                None, OP.add)
                    else:
                        nc.vector.tensor_tensor(q, in0, in1, OP.add)
                qf = q.rearrange("p a b -> p (a b)")
                for _m in range(st.get("act", 0)):
                    if lrelu_mode == "act":
                        nc.scalar.activation(qf, qf, AF.Prelu,
                                             alpha=NEG_SLOPE)
                        nc.scalar.activation(qf, qf, AF.Exp)
                    else:  # exp(leaky(x)) == max(exp(x), exp(0.2 x))
                        q2 = qp.tile([P, sc_chunks, rows], f32, name="q2",
                                     tag="q2")
                        q2f = q2.rearrange("p a b -> p (a b)")
                        nc.scalar.activation(q2f, qf, AF.Exp,
                                             scale=NEG_SLOPE)
                        nc.scalar.activation(qf, qf, AF.Exp)
                        nc.vector.tensor_tensor(qf, qf, q2f, OP.max)
                # mask multiply, split DVE / GpSimd
                dm = min(dve_mask_chunks, sc_chunks)
                for _m in range(st.get("mask", 0)):
                    nc.vector.tensor_tensor(q[:, :dm, :], q[:, :dm, :],
                                            adjT[:, :dm, :], OP.mult)
                    if dm < sc_chunks:
                        nc.gpsimd.tensor_tensor(q[:, dm:, :], q[:, dm:, :],
                                                adjT[:, dm:, :], OP.mult)
                n_mm = st.get("mm", 0)
                for _m in range(n_mm):
                    for cc in range(sc_chunks):
                        jc = sc * sc_chunks + cc
                        for h in range(n_half):
                            nc.tensor.matmul(
                                acc[h][:, :],
                                lhsT=whaug_sb[:, jc, :],
                                rhs=q[:, cc, h * 512:(h + 1) * 512],
                                start=(jc == 0 and _m == 0),
                                stop=(jc == n_chunk - 1 and _m == n_mm - 1))

            # ---- epilogue phase 1: h' + LayerNorm (T-layout) -> SBUF ----
            hs = []
            do_epi = st.get("epi", 0) > 0 and st.get("mm", 0) > 0
            for h in range(n_half if do_epi else 0):
                rec = hp.tile([1, 512], f32, tag="rec")
                nc.vector.reciprocal(rec, acc[h][64:65, :])
                rbc = hp.tile([D_H, 512], f32, tag="rbc")
                bcast_sb(rbc, rec[0:1, :], D_H)
                hT = hp.tile([D_H, 512], f32, tag="hT", bufs=n_half)
                nc.vector.tensor_tensor(hT, acc[h][0:D_H, :], rbc, OP.mult)
                sq = hp.tile([D_H, 512], f32, tag="sq")
                nc.scalar.activation(sq, hT, AF.Square)
                ssum = accp.tile([1, 512], f32, tag="ssum", name="ssum")
                nc.tensor.matmul(ssum, lhsT=ones48, rhs=hT,
                                 start=True, stop=True)
                ssq = accp.tile([1, 512], f32, tag="ssq", name="ssq")
                nc.tensor.matmul(ssq, lhsT=ones48, rhs=sq,
                                 start=True, stop=True)
                mean = hp.tile([1, 512], f32, tag="mean")
                nc.scalar.activation(mean, ssum, AF.Copy, scale=1.0 / D_H)
                var = hp.tile([1, 512], f32, tag="var")
                nc.scalar.activation(var, ssq, AF.Copy, scale=1.0 / D_H)
                msq = hp.tile([1, 512], f32, tag="msq")
                nc.vector.tensor_tensor(msq, mean, mean, OP.mult)
                nc.vector.tensor_tensor(var, var, msq, OP.subtract)
                std = hp.tile([1, 512], f32, tag="std")
                nc.scalar.activation(std, var, AF.Sqrt, bias=eps_sb[0:1, :])
                rstd = hp.tile([1, 512], f32, tag="rstd")
                nc.vector.reciprocal(rstd, std)
                mbc = hp.tile([D_H, 512], f32, tag="mbc")
                bcast_sb(mbc, mean[0:1, :], D_H)
                sbc = hp.tile([D_H, 512], f32, tag="sbc")
                bcast_sb(sbc, rstd[0:1, :], D_H)
                nc.vector.tensor_tensor(hT, hT, mbc, OP.subtract)
                nc.vector.tensor_tensor(hT, hT, sbc, OP.mult)
                nc.vector.tensor_scalar(hT, hT, gammaC, betaC,
                                        OP.mult, OP.add)
                hs.append(hT)

          # ---- epilogue phase 2: MLP head in transposed layout ----
          with tc.tile_pool(name=f"mlpp{rep}", bufs=1, space="PSUM") as mlpp:
            for h in range(n_half if do_epi else 0):
                h1 = hp.tile([P, 2, 512], f32, tag="h1")
                for m in range(2):
                    m1 = mlpp.tile([P, 512], f32, tag="m1")
                    nc.tensor.matmul(m1, lhsT=w1t_sb[:, m * P:(m + 1) * P],
                                     rhs=hs[h], start=True, stop=True)
                    nc.scalar.activation(h1[:, m, :], m1, AF.Relu,
                                         bias=b1_sb[:, m:m + 1])
                m2 = mlpp.tile([P, 512], f32, tag="m2")
                for m in range(2):
                    nc.tensor.matmul(m2, lhsT=w2t_sb[:, m, :],
                                     rhs=h1[:, m, :],
                                     start=(m == 0), stop=(m == 1))
                h2 = hp.tile([P, 512], f32, tag="h2")
                nc.scalar.activation(h2, m2, AF.Relu, bias=b2_sb)
                m3 = mlpp.tile([D_OUT, 512], f32, tag="m3")
                nc.tensor.matmul(m3, lhsT=w3t_sb, rhs=h2,
                                 start=True, stop=True)
                h3 = hp.tile([D_OUT, 512], f32, tag="h3")
                nc.scalar.activation(h3, m3, AF.Identity, bias=b3_sb)
                for k in range(4):
                    ko = h * 4 + k
                    m4 = mlpp.tile([P, D_OUT], f32, tag="m4")
                    nc.tensor.transpose(m4, h3[:, k * P:(k + 1) * P],
                                        ident[0:D_OUT, 0:D_OUT])
                    ob = hp.tile([P, D_OUT], f32, tag="ob")
                    nc.vector.tensor_copy(ob, m4)
                    nc.sync.dma_start(out_d[ko * P:(ko + 1) * P, :], ob)

    nc.compile()
    return nc


def host_prep(x, adj, W_gat, a, gamma, beta, W1, b1, W2, b2, W3, b3,
              num_cores=N_CORES):
    n = x.shape[0]
    rows = n // num_cores
    n_chunk = n // P
    Wh = (x @ W_gat.T).astype(np.float32)
    s = (Wh @ a.T).astype(np.float32).ravel()
    whaug = np.concatenate([Wh, np.zeros((n, 17), np.float32)], axis=1)
    whaug[:, 64] = 1.0
    whaug = np.ascontiguousarray(whaug)
    adj16 = adj.astype(np.int16)
    sP = np.ascontiguousarray(s.reshape(n_chunk, P).T)
    in_maps = []
    for c in range(num_cores):
        r = slice(c * rows, (c + 1) * rows)
        in_maps.append({
            "adj16": np.ascontiguousarray(adj16[r]),
            "whaug": whaug,
            "sP": sP,
            "sI": np.ascontiguousarray(s[r][None, :]),
            "gamma": np.ascontiguousarray(gamma[None, :]).astype(np.float32),
            "beta": np.ascontiguousarray(beta[None, :]).astype(np.float32),
            "w1t": np.ascontiguousarray(W1.T).astype(np.float32),
            "b1": np.ascontiguousarray(b1[:, None]).astype(np.float32),
            "w2t": np.ascontiguousarray(W2.T).astype(np.float32),
            "b2": np.ascontiguousarray(b2[:, None]).astype(np.float32),
            "w3t": np.ascontiguousarray(W3.T).astype(np.float32),
            "b3": np.ascontiguousarray(b3[:, None]).astype(np.float32),
        })
    return in_maps


_NC_CACHE = {}


def kernel(x, adj, W_gat, a, gamma, beta, W1, b1, W2, b2, W3, b3,
           trace=False):
    from concourse.bass_utils import run_bass_kernel_spmd

    args = [np.asarray(t) for t in
            (x, adj, W_gat, a, gamma, beta, W1, b1, W2, b2, W3, b3)]
    in_maps = host_prep(*args)
    if "nc" not in _NC_CACHE:
        _NC_CACHE["nc"] = build_nc()
    nc = _NC_CACHE["nc"]
    res = run_bass_kernel_spmd(nc, in_maps, list(range(N_CORES)), trace=trace)
    out = np.concatenate([r["out"] for r in res.results], axis=0)
    if trace:
        kernel.last_results = res
    return out.astype(np.float32)



# revision 8
# speedup vs baseline: 1.5110x; 1.5110x over previous
"""Trainium2 Bass kernel for nn_MetaRL_LightGAT_BiACT (GAT + LayerNorm + MLP).

Strategy (8 NeuronCores, row-sharded, indicator-split formulation):

  exp(leaky_relu(s_i + s_j)) is exactly u_i*v_j when s_i+s_j > 0 and
  w_i*z_j otherwise, where u=exp(s), w=exp(0.2 s) (v=u, z=w over j).
  With c_ij = 1[s_i+s_j > 0] and A1 = adj*c, the GAT aggregation
  numerator (and denominator, via a ones column) becomes

     num_i = u_i * (A1 @ vWh)_i + w_i * ((adj @ zWh)_i - (A1 @ zWh)_i)

  i.e. two plain matmuls per j-chunk with 0/1 rhs masks -- no exp or
  leaky-relu over the N^2 data at all.

  Host precomputes a single pre-transposed fp16 slab
     slabG[j, i] = s_i + 4*(adj[i,j] - 1)
  from which BOTH masks fall out as one tensor_scalar each (4x DVE mode):
     A1 = (slabG + s_j) > 0        (adj=0 entries are < -2, never pass)
     A0 = slabG > -2               (recovers adj)

  Per j-chunk (128 j's x 1024 i's) on each core:
    DMA:  slabG chunk [128, 1024] fp16 (plain contiguous load)
    DVE:  A1 = ts(slabG add s_j, is_gt 0) -> bf16   (4x mode)
          A0 = ts(slabG is_gt -2)         -> bf16   (4x mode)
    PE:   accCat[0:128]  += [vWh | -zWh]_chunk^T @ A1   (bf16, 1 cyc/row)
          accCat[64:128] += zWh_chunk^T @ A0            (same PSUM bank;
                            accumulates zWh@(A0-A1) in rows 64..112)
  Epilogue: shift accCat[64:113] down via tiny DMA, combine with u/w,
  divide by denominator row, LayerNorm (f32), MLP 48->256->128->32 in
  bf16 on PE, transpose out.
"""

import sys

if "/opt/trn_rl_repo" not in sys.path:
    sys.path.insert(0, "/opt/trn_rl_repo")

import numpy as np

N = 8192
D_IN = 128
D_H = 48
D_OUT = 32
N_CORES = 8
ROWS = N // N_CORES          # 1024 rows (i) per core
P = 128                      # partitions
NEG_SLOPE = 0.2
EPS = 1e-5
MOFF = 4.0                   # mask offset folded into slabG


def build_nc(num_cores=N_CORES, rows=ROWS, n=N, slab_bufs=3, mask_bufs=2,
             reps=1, stages="dma,cmp,mm,epi"):
    import concourse.bass as bass
    import concourse.mybir as mybir
    import concourse.tile as tile
    from concourse import bacc
    from concourse.masks import make_identity
    from contextlib import ExitStack

    f32 = mybir.dt.float32
    f16 = mybir.dt.float16
    bf16 = mybir.dt.bfloat16
    AF = mybir.ActivationFunctionType
    OP = mybir.AluOpType

    n_chunk = n // P             # j-chunks
    n_half = rows // 512         # 512-wide i halves

    st = {}
    for tok in stages.split(","):
        name, _, mult = tok.partition(":")
        st[name] = int(mult) if mult else 1
    nc = bacc.Bacc("TRN2", target_bir_lowering=False, debug=False,
                   num_devices=num_cores)

    slab_d = nc.dram_tensor("slabg", [n, rows], f16, kind="ExternalInput").ap()
    catwh_d = nc.dram_tensor("catwh", [n, P], bf16, kind="ExternalInput").ap()
    zwh_d = nc.dram_tensor("zwh", [n, 64], bf16, kind="ExternalInput").ap()
    sP_d = nc.dram_tensor("sP", [P, n_chunk], f32, kind="ExternalInput").ap()
    uw_d = nc.dram_tensor("uw", [2, rows], f32, kind="ExternalInput").ap()
    gamma_d = nc.dram_tensor("gamma", [1, D_H], f32, kind="ExternalInput").ap()
    beta_d = nc.dram_tensor("beta", [1, D_H], f32, kind="ExternalInput").ap()
    w1t_d = nc.dram_tensor("w1t", [D_H, 256], bf16, kind="ExternalInput").ap()
    b1_d = nc.dram_tensor("b1", [256, 1], f32, kind="ExternalInput").ap()
    w2t_d = nc.dram_tensor("w2t", [256, 128], bf16, kind="ExternalInput").ap()
    b2_d = nc.dram_tensor("b2", [128, 1], f32, kind="ExternalInput").ap()
    w3t_d = nc.dram_tensor("w3t", [128, D_OUT], bf16, kind="ExternalInput").ap()
    b3_d = nc.dram_tensor("b3", [D_OUT, 1], f32, kind="ExternalInput").ap()
    out_d = nc.dram_tensor("out", [rows, D_OUT], f32, kind="ExternalOutput").ap()

    with ExitStack() as ctx:
        tc = ctx.enter_context(tile.TileContext(nc))
        singles = ctx.enter_context(tc.tile_pool(name="singles", bufs=1))
        slabp = ctx.enter_context(tc.tile_pool(name="slabp", bufs=slab_bufs))
        maskp = ctx.enter_context(tc.tile_pool(name="maskp", bufs=mask_bufs))
        hp = ctx.enter_context(tc.tile_pool(name="hp", bufs=2))

        # ---- resident small tensors ----
        catwh_sb = singles.tile([P, n_chunk, P], bf16)
        nc.sync.dma_start(catwh_sb, catwh_d.rearrange("(c p) m -> p c m", p=P))
        zwh_sb = singles.tile([P, n_chunk, 64], bf16)
        nc.sync.dma_start(zwh_sb, zwh_d.rearrange("(c p) m -> p c m", p=P))
        sP_sb = singles.tile([P, n_chunk], f32)
        nc.sync.dma_start(sP_sb, sP_d)
        # u replicated over partitions 0..63, w over all 128 (used at 64:113)
        u_rep = singles.tile([64, rows], f32)
        nc.sync.dma_start(u_rep, uw_d[0:1, :].partition_broadcast(64)
                          .rearrange("p one r -> p (one r)"))
        w_rep = singles.tile([P, rows], f32)
        nc.sync.dma_start(w_rep, uw_d[1:2, :].partition_broadcast(P)
                          .rearrange("p one r -> p (one r)"))
        gammaC = singles.tile([D_H, 1], f32)
        nc.sync.dma_start(gammaC, gamma_d.rearrange("one d -> d one"))
        betaC = singles.tile([D_H, 1], f32)
        nc.sync.dma_start(betaC, beta_d.rearrange("one d -> d one"))
        w1t_sb = singles.tile([D_H, 256], bf16)
        nc.sync.dma_start(w1t_sb, w1t_d)
        w2t_sb = singles.tile([P, 2, 128], bf16)
        nc.sync.dma_start(w2t_sb, w2t_d.rearrange("(m p) k -> p m k", p=P))
        w3t_sb = singles.tile([P, D_OUT], bf16)
        nc.sync.dma_start(w3t_sb, w3t_d)
        b1_sb = singles.tile([P, 2], f32)
        nc.sync.dma_start(b1_sb, b1_d.rearrange("(m p) one -> p (m one)", p=P))
        b2_sb = singles.tile([P, 1], f32)
        nc.sync.dma_start(b2_sb, b2_d)
        b3_sb = singles.tile([D_OUT, 1], f32)
        nc.sync.dma_start(b3_sb, b3_d)
        eps_sb = singles.tile([P, 1], f32)
        nc.vector.memset(eps_sb, EPS)
        ones48 = singles.tile([D_H, 1], f32)
        nc.vector.memset(ones48, 1.0)
        ident = singles.tile([P, P], f32)
        make_identity(nc, ident)

        def bcast_sb(dst, src_row, parts, eng=None):
            src = bass.AP(tensor=src_row.tensor, offset=src_row.offset,
                          ap=[src_row.ap[0], [0, parts], src_row.ap[1]])
            dst3 = bass.AP(tensor=dst.tensor, offset=dst.offset,
                           ap=[dst.ap[0], [1, 1], dst.ap[1]])
            (eng or nc.sync).dma_start(dst3, src)

        slab_r = slab_d.rearrange("(c p) i -> p c i", p=P)

        for rep in range(reps):
          with tc.tile_pool(name=f"accp{rep}", bufs=2, space="PSUM") as accp:
            acc = [accp.tile([P, 512], f32, tag="acc", name=f"acc{h}")
                   for h in range(n_half)]
            for cc in range(n_chunk):
                slab = slabp.tile([P, rows], f16, tag="slab")
                for _m in range(st.get("dma", 0)):
                    nc.sync.dma_start(slab, slab_r[:, cc, :])
                a1 = maskp.tile([P, rows], bf16, tag="a1")
                a0 = maskp.tile([P, rows], bf16, tag="a0")
                for _m in range(st.get("cmp", 0)):
                    nc.vector.tensor_scalar(a1, slab, sP_sb[:, cc:cc + 1],
                                            0.0, OP.add, OP.is_gt)
                    nc.vector.tensor_scalar(a0, slab, -(MOFF / 2), None,
                                            OP.is_gt)
                n_mm = st.get("mm", 0)
                for _m in range(n_mm):
                    for h in range(n_half):
                        sl = slice(h * 512, (h + 1) * 512)
                        nc.tensor.matmul(
                            acc[h][:, :],
                            lhsT=catwh_sb[:, cc, :],
                            rhs=a1[:, sl],
                            start=(cc == 0 and _m == 0), stop=False,
                            skip_group_check=True)
                    for h in range(n_half):
                        sl = slice(h * 512, (h + 1) * 512)
                        nc.tensor.matmul(
                            acc[h][64:128, :],
                            lhsT=zwh_sb[:, cc, :],
                            rhs=a0[:, sl],
                            start=False,
                            stop=(cc == n_chunk - 1 and _m == n_mm - 1),
                            skip_group_check=True)

            # ---- epilogue phase 1: combine, divide, LayerNorm ----
            hs = []
            do_epi = st.get("epi", 0) > 0 and st.get("mm", 0) > 0
            for h in range(n_half if do_epi else 0):
                sl = slice(h * 512, (h + 1) * 512)
                # combine: acc rows 0:49 hold vWh@A1 (u side), rows 64:113
                # hold zWh@(A0-A1) (w side). Weight each in place in PSUM,
                # bounce to SBUF, shift the w side down 64 partitions via
                # DMA, add, then divide by the denominator row.
                nc.vector.tensor_tensor(acc[h][64:113, :], acc[h][64:113, :],
                                        w_rep[64:113, sl], OP.mult)
                nc.vector.tensor_tensor(acc[h][0:49, :], acc[h][0:49, :],
                                        u_rep[0:49, sl], OP.mult)
                comb = hp.tile([P, 512], f32, tag="comb")
                nc.scalar.activation(comb, acc[h][:, :], AF.Copy)
                nb = hp.tile([49, 512], f32, tag="nb")
                nc.scalar.dma_start(nb, comb[64:113, :])
                numT = hp.tile([49, 512], f32, tag="numT")
                nc.vector.tensor_tensor(numT, comb[0:49, :], nb, OP.add)
                den0 = hp.tile([1, 512], f32, tag="den0")
                nc.scalar.dma_start(den0, numT[48:49, :])
                rec = hp.tile([1, 512], f32, tag="rec")
                nc.vector.reciprocal(rec, den0)
                rbc = hp.tile([D_H, 512], f32, tag="rbc")
                bcast_sb(rbc, rec[0:1, :], D_H, eng=nc.scalar)
                hT = hp.tile([D_H, 512], f32, tag="hT", bufs=n_half)
                nc.vector.tensor_tensor(hT, numT[0:D_H, :], rbc, OP.mult)
                sq = hp.tile([D_H, 512], f32, tag="sq")
                nc.scalar.activation(sq, hT, AF.Square)
                ssum = accp.tile([1, 512], f32, tag="ssum", name="ssum")
                nc.tensor.matmul(ssum, lhsT=ones48, rhs=hT,
                                 start=True, stop=True)
                ssq = accp.tile([1, 512], f32, tag="ssq", name="ssq")
                nc.tensor.matmul(ssq, lhsT=ones48, rhs=sq,
                                 start=True, stop=True)
                mean = hp.tile([1, 512], f32, tag="mean")
                nc.scalar.activation(mean, ssum, AF.Copy, scale=1.0 / D_H)
                var = hp.tile([1, 512], f32, tag="var")
                nc.scalar.activation(var, ssq, AF.Copy, scale=1.0 / D_H)
                msq = hp.tile([1, 512], f32, tag="msq")
                nc.vector.tensor_tensor(msq, mean, mean, OP.mult)
                nc.vector.tensor_tensor(var, var, msq, OP.subtract)
                std = hp.tile([1, 512], f32, tag="std")
                nc.scalar.activation(std, var, AF.Sqrt, bias=eps_sb[0:1, :])
                rstd = hp.tile([1, 512], f32, tag="rstd")
                nc.vector.reciprocal(rstd, std)
                mbc = hp.tile([D_H, 512], f32, tag="mbc")
                bcast_sb(mbc, mean[0:1, :], D_H, eng=nc.scalar)
                sbc = hp.tile([D_H, 512], f32, tag="sbc")
                bcast_sb(sbc, rstd[0:1, :], D_H, eng=nc.scalar)
                nc.vector.tensor_tensor(hT, hT, mbc, OP.subtract)
                nc.vector.tensor_tensor(hT, hT, sbc, OP.mult)
                hTb = hp.tile([D_H, 512], bf16, tag="hTb", bufs=n_half)
                nc.vector.tensor_scalar(hTb, hT, gammaC, betaC,
                                        OP.mult, OP.add)
                hs.append(hTb)

          # ---- epilogue phase 2: MLP head in transposed layout (bf16) ----
          with tc.tile_pool(name=f"mlpp{rep}", bufs=1, space="PSUM") as mlpp:
            for h in range(n_half if do_epi else 0):
                h1 = hp.tile([P, 2, 512], bf16, tag="h1")
                for m in range(2):
                    m1 = mlpp.tile([P, 512], f32, tag="m1")
                    nc.tensor.matmul(m1, lhsT=w1t_sb[:, m * P:(m + 1) * P],
                                     rhs=hs[h], start=True, stop=True)
                    nc.scalar.activation(h1[:, m, :], m1, AF.Relu,
                                         bias=b1_sb[:, m:m + 1])
                m2 = mlpp.tile([P, 512], f32, tag="m2")
                for m in range(2):
                    nc.tensor.matmul(m2, lhsT=w2t_sb[:, m, :],
                                     rhs=h1[:, m, :],
                                     start=(m == 0), stop=(m == 1))
                h2 = hp.tile([P, 512], bf16, tag="h2")
                nc.scalar.activation(h2, m2, AF.Relu, bias=b2_sb)
                m3 = mlpp.tile([D_OUT, 512], f32, tag="m3")
                nc.tensor.matmul(m3, lhsT=w3t_sb, rhs=h2,
                                 start=True, stop=True)
                h3 = hp.tile([D_OUT, 512], f32, tag="h3")
                nc.scalar.activation(h3, m3, AF.Identity, bias=b3_sb)
                for k in range(4):
                    ko = h * 4 + k
                    m4 = mlpp.tile([P, D_OUT], f32, tag="m4")
                    nc.tensor.transpose(m4, h3[:, k * P:(k + 1) * P],
                                        ident[0:D_OUT, 0:D_OUT])
                    ob = hp.tile([P, D_OUT], f32, tag="ob")
                    nc.vector.tensor_copy(ob, m4)
                    nc.scalar.dma_start(out_d[ko * P:(ko + 1) * P, :], ob)

    nc.compile()
    return nc


def host_prep(x, adj, W_gat, a, gamma, beta, W1, b1, W2, b2, W3, b3,
              num_cores=N_CORES):
    import ml_dtypes

    bf16 = ml_dtypes.bfloat16
    n = x.shape[0]
    rows = n // num_cores
    n_chunk = n // P
    Wh = (x.astype(np.float32) @ W_gat.T.astype(np.float32))
    s = (Wh @ a.T.astype(np.float32)).ravel().astype(np.float32)
    assert np.abs(s).max() < MOFF / 2 - 0.1, "s out of slab-offset range"
    u = np.exp(s).astype(np.float32)          # exp(s)
    w = np.exp(NEG_SLOPE * s).astype(np.float32)
    # catwh: [vWh(48) v 0*15 | -zWh(48) -z 0*15]
    catwh = np.zeros((n, P), np.float32)
    catwh[:, 0:D_H] = u[:, None] * Wh
    catwh[:, D_H] = u
    catwh[:, 64:64 + D_H] = -(w[:, None] * Wh)
    catwh[:, 64 + D_H] = -w
    zwh = np.zeros((n, 64), np.float32)
    zwh[:, 0:D_H] = w[:, None] * Wh
    zwh[:, D_H] = w
    sP = np.ascontiguousarray(s.reshape(n_chunk, P).T)
    in_maps = []
    for c in range(num_cores):
        r = slice(c * rows, (c + 1) * rows)
        slabg = (s[r][None, :] +
                 MOFF * (adj[r].T.astype(np.float32) - 1.0)
                 ).astype(np.float16)
        in_maps.append({
            "slabg": np.ascontiguousarray(slabg),
            "catwh": catwh.astype(bf16),
            "zwh": zwh.astype(bf16),
            "sP": sP,
            "uw": np.ascontiguousarray(np.stack([u[r], w[r]])),
            "gamma": np.ascontiguousarray(gamma[None, :]).astype(np.float32),
            "beta": np.ascontiguousarray(beta[None, :]).astype(np.float32),
            "w1t": np.ascontiguousarray(W1.T).astype(bf16),
            "b1": np.ascontiguousarray(b1[:, None]).astype(np.float32),
            "w2t": np.ascontiguousarray(W2.T).astype(bf16),
            "b2": np.ascontiguousarray(b2[:, None]).astype(np.float32),
            "w3t": np.ascontiguousarray(W3.T).astype(bf16),
            "b3": np.ascontiguousarray(b3[:, None]).astype(np.float32),
        })
    return in_maps


_NC_CACHE = {}


def kernel(x, adj, W_gat, a, gamma, beta, W1, b1, W2, b2, W3, b3,
           trace=False):
    from concourse.bass_utils import run_bass_kernel_spmd

    args = [np.asarray(t) for t in
            (x, adj, W_gat, a, gamma, beta, W1, b1, W2, b2, W3, b3)]
    in_maps = host_prep(*args)
    if "nc" not in _NC_CACHE:
        _NC_CACHE["nc"] = build_nc()
    nc = _NC_CACHE["nc"]
    res = run_bass_kernel_spmd(nc, in_maps, list(range(N_CORES)), trace=trace)
    out = np.concatenate([r["out"] for r in res.results], axis=0)
    if trace:
        kernel.last_results = res
    return out.astype(np.float32)


# revision 9
# speedup vs baseline: 1.7521x; 1.1596x over previous
"""Trainium2 Bass kernel for nn_MetaRL_LightGAT_BiACT (GAT + LayerNorm + MLP).

Strategy (8 NeuronCores, row-sharded, indicator-split formulation):

  exp(leaky_relu(s_i + s_j)) is exactly u_i*v_j when s_i+s_j > 0 and
  w_i*z_j otherwise, where u=exp(s), w=exp(0.2 s) (v=u, z=w over j).
  With c_ij = 1[s_i+s_j > 0] and A1 = adj*c, the GAT aggregation
  numerator (and denominator, via a ones column) becomes

     num_i = u_i * (A1 @ vWh)_i + w_i * ((adj @ zWh)_i - (A1 @ zWh)_i)

  i.e. two plain matmuls per j-chunk with 0/1 rhs masks -- no exp or
  leaky-relu over the N^2 data at all.

  Host precomputes a single pre-transposed fp16 slab
     slabG[j, i] = s_i + 4*(adj[i,j] - 1)
  from which BOTH masks fall out as one tensor_scalar each (4x DVE mode):
     A1 = (slabG + s_j) > 0        (adj=0 entries are < -2, never pass)
     A0 = slabG > -2               (recovers adj)

  Per j-chunk (128 j's x 1024 i's) on each core:
    DMA:  slabG chunk [128, 1024] fp16 (plain contiguous load)
    DVE:  A1 = ts(slabG add s_j, is_gt 0) -> bf16   (4x mode)
          A0 = ts(slabG is_gt -2)         -> bf16   (4x mode)
    PE:   accCat[0:128]  += [vWh | -zWh]_chunk^T @ A1   (bf16, 1 cyc/row)
          accCat[64:128] += zWh_chunk^T @ A0            (same PSUM bank;
                            accumulates zWh@(A0-A1) in rows 64..112)
  Epilogue: shift accCat[64:113] down via tiny DMA, combine with u/w,
  divide by denominator row, LayerNorm (f32), MLP 48->256->128->32 in
  bf16 on PE, transpose out.
"""

import sys

if "/opt/trn_rl_repo" not in sys.path:
    sys.path.insert(0, "/opt/trn_rl_repo")

import numpy as np

N = 8192
D_IN = 128
D_H = 48
D_OUT = 32
N_CORES = 8
ROWS = N // N_CORES          # 1024 rows (i) per core
P = 128                      # partitions
NEG_SLOPE = 0.2
EPS = 1e-5
MOFF = 4.0                   # mask offset folded into slabG


def build_nc(num_cores=N_CORES, rows=ROWS, n=N, slab_bufs=3, mask_bufs=2,
             reps=1, stages="dma,cmp,mm,epi"):
    import concourse.bass as bass
    import concourse.mybir as mybir
    import concourse.tile as tile
    from concourse import bacc
    from concourse.masks import make_identity
    from contextlib import ExitStack

    f32 = mybir.dt.float32
    f16 = mybir.dt.float16
    bf16 = mybir.dt.bfloat16
    AF = mybir.ActivationFunctionType
    OP = mybir.AluOpType

    n_chunk = n // P             # j-chunks
    n_half = rows // 512         # 512-wide i halves

    st = {}
    for tok in stages.split(","):
        name, _, mult = tok.partition(":")
        st[name] = int(mult) if mult else 1
    nc = bacc.Bacc("TRN2", target_bir_lowering=False, debug=False,
                   num_devices=num_cores)

    slab_d = nc.dram_tensor("slabg", [n, rows], bf16, kind="ExternalInput").ap()
    catwh_d = nc.dram_tensor("catwh", [n, P], bf16, kind="ExternalInput").ap()
    zwh_d = nc.dram_tensor("zwh", [n, 64], bf16, kind="ExternalInput").ap()
    sP_d = nc.dram_tensor("sP", [P, n_chunk], f32, kind="ExternalInput").ap()
    uw_d = nc.dram_tensor("uw", [2, rows], f32, kind="ExternalInput").ap()
    gamma_d = nc.dram_tensor("gamma", [1, D_H], f32, kind="ExternalInput").ap()
    beta_d = nc.dram_tensor("beta", [1, D_H], f32, kind="ExternalInput").ap()
    w1t_d = nc.dram_tensor("w1t", [D_H, 256], bf16, kind="ExternalInput").ap()
    b1_d = nc.dram_tensor("b1", [256, 1], f32, kind="ExternalInput").ap()
    w2t_d = nc.dram_tensor("w2t", [256, 128], bf16, kind="ExternalInput").ap()
    b2_d = nc.dram_tensor("b2", [128, 1], f32, kind="ExternalInput").ap()
    w3t_d = nc.dram_tensor("w3t", [128, D_OUT], bf16, kind="ExternalInput").ap()
    b3_d = nc.dram_tensor("b3", [D_OUT, 1], f32, kind="ExternalInput").ap()
    out_d = nc.dram_tensor("out", [rows, D_OUT], f32, kind="ExternalOutput").ap()

    with ExitStack() as ctx:
        tc = ctx.enter_context(tile.TileContext(nc))
        singles = ctx.enter_context(tc.tile_pool(name="singles", bufs=1))
        slabp = ctx.enter_context(tc.tile_pool(name="slabp", bufs=slab_bufs))
        maskp = ctx.enter_context(tc.tile_pool(name="maskp", bufs=mask_bufs))
        hp = ctx.enter_context(tc.tile_pool(name="hp", bufs=2))

        # ---- resident small tensors ----
        catwh_sb = singles.tile([P, n_chunk, P], bf16)
        nc.sync.dma_start(catwh_sb, catwh_d.rearrange("(c p) m -> p c m", p=P))
        zwh_sb = singles.tile([P, n_chunk, 64], bf16)
        nc.sync.dma_start(zwh_sb, zwh_d.rearrange("(c p) m -> p c m", p=P))
        sP_sb = singles.tile([P, n_chunk], f32)
        nc.sync.dma_start(sP_sb, sP_d)
        # u replicated over partitions 0..63, w over all 128 (used at 64:113)
        u_rep = singles.tile([64, rows], f32)
        nc.sync.dma_start(u_rep, uw_d[0:1, :].partition_broadcast(64)
                          .rearrange("p one r -> p (one r)"))
        w_rep = singles.tile([P, rows], f32)
        nc.sync.dma_start(w_rep, uw_d[1:2, :].partition_broadcast(P)
                          .rearrange("p one r -> p (one r)"))
        gammaC = singles.tile([D_H, 1], f32)
        nc.sync.dma_start(gammaC, gamma_d.rearrange("one d -> d one"))
        betaC = singles.tile([D_H, 1], f32)
        nc.sync.dma_start(betaC, beta_d.rearrange("one d -> d one"))
        w1t_sb = singles.tile([D_H, 256], bf16)
        nc.sync.dma_start(w1t_sb, w1t_d)
        w2t_sb = singles.tile([P, 2, 128], bf16)
        nc.sync.dma_start(w2t_sb, w2t_d.rearrange("(m p) k -> p m k", p=P))
        w3t_sb = singles.tile([P, D_OUT], bf16)
        nc.sync.dma_start(w3t_sb, w3t_d)
        b1_sb = singles.tile([P, 2], f32)
        nc.sync.dma_start(b1_sb, b1_d.rearrange("(m p) one -> p (m one)", p=P))
        b2_sb = singles.tile([P, 1], f32)
        nc.sync.dma_start(b2_sb, b2_d)
        b3_sb = singles.tile([D_OUT, 1], f32)
        nc.sync.dma_start(b3_sb, b3_d)
        eps_sb = singles.tile([P, 1], f32)
        nc.vector.memset(eps_sb, EPS)
        ones48 = singles.tile([D_H, 1], bf16)
        nc.vector.memset(ones48, 1.0)
        ident = singles.tile([P, P], f32)
        make_identity(nc, ident)

        def bcast_sb(dst, src_row, parts, eng=None):
            src = bass.AP(tensor=src_row.tensor, offset=src_row.offset,
                          ap=[src_row.ap[0], [0, parts], src_row.ap[1]])
            dst3 = bass.AP(tensor=dst.tensor, offset=dst.offset,
                           ap=[dst.ap[0], [1, 1], dst.ap[1]])
            (eng or nc.sync).dma_start(dst3, src)

        slab_r = slab_d.rearrange("(c p) i -> p c i", p=P)

        for rep in range(reps):
          with tc.tile_pool(name=f"accp{rep}", bufs=2, space="PSUM") as accp:
            acc = [accp.tile([P, 512], f32, tag="acc", name=f"acc{h}")
                   for h in range(n_half)]
            for cc in range(n_chunk):
                slab = slabp.tile([P, rows], bf16, tag="slab")
                for _m in range(st.get("dma", 0)):
                    nc.sync.dma_start(slab, slab_r[:, cc, :])
                a1 = maskp.tile([P, rows], bf16, tag="a1")
                a0 = maskp.tile([P, rows], bf16, tag="a0")
                for _m in range(st.get("cmp", 0)):
                    nc.vector.tensor_scalar(a1, slab, sP_sb[:, cc:cc + 1],
                                            0.0, OP.add, OP.is_gt)
                    nc.vector.tensor_scalar(a0, slab, -(MOFF / 2), None,
                                            OP.is_gt)
                n_mm = st.get("mm", 0)
                for _m in range(n_mm):
                    for h in range(n_half):
                        sl = slice(h * 512, (h + 1) * 512)
                        nc.tensor.matmul(
                            acc[h][:, :],
                            lhsT=catwh_sb[:, cc, :],
                            rhs=a1[:, sl],
                            start=(cc == 0 and _m == 0), stop=False,
                            skip_group_check=True)
                    for h in range(n_half):
                        sl = slice(h * 512, (h + 1) * 512)
                        nc.tensor.matmul(
                            acc[h][64:128, :],
                            lhsT=zwh_sb[:, cc, :],
                            rhs=a0[:, sl],
                            start=False,
                            stop=(cc == n_chunk - 1 and _m == n_mm - 1),
                            skip_group_check=True)

            # ---- epilogue phase 1: combine, divide, LayerNorm ----
            hs = []
            do_epi = st.get("epi", 0) > 0 and st.get("mm", 0) > 0
            for h in range(n_half if do_epi else 0):
                sl = slice(h * 512, (h + 1) * 512)
                # combine: acc rows 0:49 hold vWh@A1 (u side), rows 64:113
                # hold zWh@(A0-A1) (w side). Weight each in place in PSUM,
                # bounce to SBUF, shift the w side down 64 partitions via
                # DMA, add, then divide by the denominator row.
                nc.vector.tensor_tensor(acc[h][64:113, :], acc[h][64:113, :],
                                        w_rep[64:113, sl], OP.mult)
                nc.vector.tensor_tensor(acc[h][0:49, :], acc[h][0:49, :],
                                        u_rep[0:49, sl], OP.mult)
                comb = hp.tile([P, 512], f32, tag="comb")
                nc.scalar.activation(comb, acc[h][:, :], AF.Copy)
                nb = hp.tile([49, 512], f32, tag="nb")
                nc.scalar.dma_start(nb, comb[64:113, :])
                numT = hp.tile([49, 512], f32, tag="numT")
                nc.vector.tensor_tensor(numT, comb[0:49, :], nb, OP.add)
                den0 = hp.tile([1, 512], f32, tag="den0")
                nc.scalar.dma_start(den0, numT[48:49, :])
                rec = hp.tile([1, 512], f32, tag="rec")
                nc.vector.reciprocal_approx_fast(rec, den0)
                rbc = hp.tile([D_H, 512], f32, tag="rbc")
                bcast_sb(rbc, rec[0:1, :], D_H, eng=nc.scalar)
                hT = hp.tile([D_H, 512], f32, tag="hT", bufs=n_half)
                nc.vector.tensor_tensor(hT, numT[0:D_H, :], rbc, OP.mult)
                hT16 = hp.tile([D_H, 512], bf16, tag="hT16")
                nc.vector.tensor_copy(hT16, hT)
                sq = hp.tile([D_H, 512], bf16, tag="sq")
                nc.scalar.activation(sq, hT16, AF.Square)
                ssum = accp.tile([1, 512], f32, tag="ssum", name="ssum")
                nc.tensor.matmul(ssum, lhsT=ones48, rhs=hT16,
                                 start=True, stop=True)
                ssq = accp.tile([1, 512], f32, tag="ssq", name="ssq")
                nc.tensor.matmul(ssq, lhsT=ones48, rhs=sq,
                                 start=True, stop=True)
                mean = hp.tile([1, 512], f32, tag="mean")
                nc.scalar.activation(mean, ssum, AF.Copy, scale=1.0 / D_H)
                var = hp.tile([1, 512], f32, tag="var")
                nc.scalar.activation(var, ssq, AF.Copy, scale=1.0 / D_H)
                msq = hp.tile([1, 512], f32, tag="msq")
                nc.vector.tensor_tensor(msq, mean, mean, OP.mult)
                nc.vector.tensor_tensor(var, var, msq, OP.subtract)
                std = hp.tile([1, 512], f32, tag="std")
                nc.scalar.activation(std, var, AF.Sqrt, bias=eps_sb[0:1, :])
                rstd = hp.tile([1, 512], f32, tag="rstd")
                nc.vector.reciprocal_approx_fast(rstd, std)
                mbc = hp.tile([D_H, 512], f32, tag="mbc")
                bcast_sb(mbc, mean[0:1, :], D_H, eng=nc.scalar)
                sbc = hp.tile([D_H, 512], f32, tag="sbc")
                bcast_sb(sbc, rstd[0:1, :], D_H, eng=nc.scalar)
                nc.vector.tensor_tensor(hT, hT, mbc, OP.subtract)
                nc.vector.tensor_tensor(hT, hT, sbc, OP.mult)
                hTb = hp.tile([D_H, 512], bf16, tag="hTb", bufs=n_half)
                nc.vector.tensor_scalar(hTb, hT, gammaC, betaC,
                                        OP.mult, OP.add)
                hs.append(hTb)

          # ---- epilogue phase 2: MLP head in transposed layout (bf16) ----
          with tc.tile_pool(name=f"mlpp{rep}", bufs=1, space="PSUM") as mlpp:
            for h in range(n_half if do_epi else 0):
                h1 = hp.tile([P, 2, 512], bf16, tag="h1")
                for m in range(2):
                    m1 = mlpp.tile([P, 512], f32, tag="m1")
                    nc.tensor.matmul(m1, lhsT=w1t_sb[:, m * P:(m + 1) * P],
                                     rhs=hs[h], start=True, stop=True)
                    nc.scalar.activation(h1[:, m, :], m1, AF.Relu,
                                         bias=b1_sb[:, m:m + 1])
                m2 = mlpp.tile([P, 512], f32, tag="m2")
                for m in range(2):
                    nc.tensor.matmul(m2, lhsT=w2t_sb[:, m, :],
                                     rhs=h1[:, m, :],
                                     start=(m == 0), stop=(m == 1))
                h2 = hp.tile([P, 512], bf16, tag="h2")
                nc.scalar.activation(h2, m2, AF.Relu, bias=b2_sb)
                m3 = mlpp.tile([D_OUT, 512], f32, tag="m3")
                nc.tensor.matmul(m3, lhsT=w3t_sb, rhs=h2,
                                 start=True, stop=True)
                h3 = hp.tile([D_OUT, 512], f32, tag="h3")
                nc.scalar.activation(h3, m3, AF.Identity, bias=b3_sb)
                for k in range(4):
                    ko = h * 4 + k
                    m4 = mlpp.tile([P, D_OUT], f32, tag="m4")
                    nc.tensor.transpose(m4, h3[:, k * P:(k + 1) * P],
                                        ident[0:D_OUT, 0:D_OUT])
                    ob = hp.tile([P, D_OUT], f32, tag="ob")
                    nc.vector.tensor_copy(ob, m4)
                    nc.scalar.dma_start(out_d[ko * P:(ko + 1) * P, :], ob)

    nc.compile()
    return nc


def host_prep(x, adj, W_gat, a, gamma, beta, W1, b1, W2, b2, W3, b3,
              num_cores=N_CORES):
    import ml_dtypes

    bf16 = ml_dtypes.bfloat16
    n = x.shape[0]
    rows = n // num_cores
    n_chunk = n // P
    Wh = (x.astype(np.float32) @ W_gat.T.astype(np.float32))
    s = (Wh @ a.T.astype(np.float32)).ravel().astype(np.float32)
    assert np.abs(s).max() < MOFF / 2 - 0.1, "s out of slab-offset range"
    u = np.exp(s).astype(np.float32)          # exp(s)
    w = np.exp(NEG_SLOPE * s).astype(np.float32)
    # catwh: [vWh(48) v 0*15 | -zWh(48) -z 0*15]
    catwh = np.zeros((n, P), np.float32)
    catwh[:, 0:D_H] = u[:, None] * Wh
    catwh[:, D_H] = u
    catwh[:, 64:64 + D_H] = -(w[:, None] * Wh)
    catwh[:, 64 + D_H] = -w
    zwh = np.zeros((n, 64), np.float32)
    zwh[:, 0:D_H] = w[:, None] * Wh
    zwh[:, D_H] = w
    sP = np.ascontiguousarray(s.reshape(n_chunk, P).T)
    in_maps = []
    for c in range(num_cores):
        r = slice(c * rows, (c + 1) * rows)
        slabg = (s[r][None, :] +
                 MOFF * (adj[r].T.astype(np.float32) - 1.0)
                 ).astype(bf16)
        in_maps.append({
            "slabg": np.ascontiguousarray(slabg),
            "catwh": catwh.astype(bf16),
            "zwh": zwh.astype(bf16),
            "sP": sP,
            "uw": np.ascontiguousarray(np.stack([u[r], w[r]])),
            "gamma": np.ascontiguousarray(gamma[None, :]).astype(np.float32),
            "beta": np.ascontiguousarray(beta[None, :]).astype(np.float32),
            "w1t": np.ascontiguousarray(W1.T).astype(bf16),
            "b1": np.ascontiguousarray(b1[:, None]).astype(np.float32),
            "w2t": np.ascontiguousarray(W2.T).astype(bf16),
            "b2": np.ascontiguousarray(b2[:, None]).astype(np.float32),
            "w3t": np.ascontiguousarray(W3.T).astype(bf16),
            "b3": np.ascontiguousarray(b3[:, None]).astype(np.float32),
        })
    return in_maps


_NC_CACHE = {}


def kernel(x, adj, W_gat, a, gamma, beta, W1, b1, W2, b2, W3, b3,
           trace=False):
    from concourse.bass_utils import run_bass_kernel_spmd

    args = [np.asarray(t) for t in
            (x, adj, W_gat, a, gamma, beta, W1, b1, W2, b2, W3, b3)]
    in_maps = host_prep(*args)
    if "nc" not in _NC_CACHE:
        _NC_CACHE["nc"] = build_nc()
    nc = _NC_CACHE["nc"]
    res = run_bass_kernel_spmd(nc, in_maps, list(range(N_CORES)), trace=trace)
    out = np.concatenate([r["out"] for r in res.results], axis=0)
    if trace:
        kernel.last_results = res
    return out.astype(np.float32)
